# revision 18
# baseline (speedup 1.0000x reference)
"""KAN layer kernel for Trainium2, data-parallel over 8 NeuronCores.

Math: out[b,o] = sum_i comb_w[i,o] * (w1*x + w2*x^2 + w3*x^3 + edge_b)[b,i,o]
    = x @ W1 + x^2 @ W2 + x^3 @ W3 + bias
  where Wp[i,o] = edge_w[i,o,p] * comb_w[i,o],  bias[o] = sum_i comb_w[i,o]*edge_b[i,o].

Sharding: batch 8-way (1024 rows/core), weights replicated.

Precision: hybrid. x/x^2 terms via fp8-e4m3 DoubleRow (a (W1,W2)x(x,x^2)
pair contracts 256 deep per instruction); the error-dominant x^3 term in
bf16; weights pre-scaled by S=256 (host), descaled in the output copy;
PSUM accumulates fp32. Measured max rel err ~1.13e-2 (gate 2e-2).

All operand prep happens on the HOST (numpy): the fp8 (x,x^2) pair planes
and bf16 x^3 plane are packed, per k-tile and batch-half, into one byte
tensor (apk); W3 (bf16) and the (W1,W2) fp8 pairs pack into another (wpk).
On-device the matmuls read them through bitcast views, so nothing sits
between "bytes land" and "matmul issues".

Schedule (per core):
- Input DMAs are issued BEFORE the TileContext (the context prologue
  otherwise delays the first issue by ~1.3us) and split across both HWDGE
  queues, ordered so each piece lands just before its first consumer:
  sync: w(t0), w(t1), acts(t1), w(t2-3); scalar: bias, acts(t0,h0),
  acts(t0,h1), acts(t2), acts(t3). Early pieces are small (the DMA path
  ramps from ~150 GB/s to ~390 GB/s over the first ~4us). Consumers hang
  off manual sync deps (the tensors are raw SBUF, not pool tiles, so
  cross-queue pieces never serialize on tile bookkeeping).
- ~2us of dummy matmuls bridge the PE from the preamble to the first
  operands so the HAM clock gate stays at 8/8 (2.4 GHz); an idle gap
  >~3us here re-throttles the PE to half clock for several us (measured).
- 64 matmuls, one dense stream, 8 PSUM banks = (batch half, o-tile).
  k-tiles 0/1 t-major, ordered to match DMA arrival (p3[h0], dr[h0],
  p3[h1], dr[h1] for t0); k-tiles 2/3 o-major so each bank finishes early
  and drains (DVE descale+bias -> bf16, then y DMA) under the next
  o-group's matmuls. The last o-tile drains h0 while the final matmul
  (h1) runs, h1 immediately after, on separate queues.
"""

import sys

import numpy as np
import ml_dtypes

sys.path.insert(0, "/opt/trn_rl_repo")

import concourse.bass as bass
import concourse.tile as tile
from concourse import bass_utils, mybir
from concourse.tile_rust import add_dep_helper

B, I, O = 8192, 512, 512
NCORES = 8
BS = B // NCORES  # 1024 rows per core
PT = 4  # 128-row tiles in I (k-tiles) and O (o-tiles)
SCALE = 256.0  # fp8/bf16 weight pre-scale (host); descaled in the output copy

BF = mybir.dt.bfloat16
F8 = mybir.dt.float8e4
F32 = mybir.dt.float32

_nc = None


def _build():
    # All HBM tensors are partition-major (leading dim 128 = SBUF partition).
    # wpk[q, t, 0] = W3 row bytes (512 bf16 as [2,512] u8); wpk[q, t, 1, p] =
    # fp8 Wp pair plane. apk[q, t, h, 0, p] = fp8 (x, x^2) pair planes;
    # apk[q, t, h, 1] = x^3 bytes (512 bf16 as [2,512] u8).
    nc = bass.Bass("TRN2", target_bir_lowering=False, debug=False)
    wpkt = nc.dram_tensor("wpk", [128, PT, 2, 2, 512], F8, kind="ExternalInput")
    apkt = nc.dram_tensor("apk", [128, PT, 2, 2, 2, 512], F8, kind="ExternalInput")
    wbt = nc.dram_tensor("wb", [128, 16], F8, kind="ExternalInput")
    yt = nc.dram_tensor("yt", [128, PT, BS], BF, kind="ExternalOutput")
    wpk_r, apk_r, wb_r, yt_r = wpkt.ap(), apkt.ap(), wbt.ap(), yt.ap()

    chains = {}

    def chain(key, inst):
        prev = chains.get(key)
        if prev is not None:
            add_dep_helper(inst.ins, prev.ins, sync=False, reason=f"{key} order")
        chains[key] = inst
        return inst

    pe = lambda inst: chain("pe", inst)
    dve = lambda inst: chain("dve", inst)

    # Raw SBUF operand buffers (not pool tiles): consumers are wired up with
    # explicit deps below, so DMA pieces never serialize on tile bookkeeping.
    wpk_sb = nc.alloc_sbuf_tensor("wpk_sb", [128, PT, 2, 2, 512], F8)
    apk_sb = nc.alloc_sbuf_tensor("apk_sb", [128, PT, 2, 2, 2, 512], F8)
    wb_sb = nc.alloc_sbuf_tensor("wb_sb", [128, 16], F8)
    wpk_a, apk_a, wb_a = wpk_sb.ap(), apk_sb.ap(), wb_sb.ap()

    # HAM warm-up: dummy matmuls on garbage SBUF bridge the PE from the
    # engine preamble (~6.9us) to the first operands (~8.5us) so the clock
    # gate is 8/8 when real matmuls start. Scratch PSUM is freed before the
    # pools allocate; real banks are zeroed by their start=True matmuls.
    warm_w = nc.alloc_sbuf_tensor("warm_w", [128, 2], BF)
    warm_x = nc.alloc_sbuf_tensor("warm_x", [128, 512], BF)
    with nc.psum_tensor("warm_ps", [128, 512], F32) as wps:
        for i in range(5):
            nc.tensor.matmul(
                wps.ap()[0:2, :], warm_w.ap(), warm_x.ap(), start=True, stop=True
            )
        for i in range(6):
            nc.tensor.matmul(
                wps.ap()[0:2, 0:128],
                warm_w.ap(),
                warm_x.ap()[:, 0:128],
                start=True,
                stop=True,
            )

    # The two pieces gating the first matmuls go out BEFORE the tile
    # context (the context prologue delays the first in-context issue by
    # ~1.4us, and the DMA path ramps slowly — start it early). They carry
    # an explicit completion semaphore; a PE-queue wait after the warmup
    # matmuls gates every real matmul behind both pieces (PE executes its
    # queue in order).
    dmas = {}
    pre_sem = nc.alloc_semaphore("pre_dma")
    nc.sync.sem_clear(pre_sem)
    # Smallest possible first pieces — just the W3(t0) and x^3(t0,h0)
    # planes the first four matmuls read (128 KB each; the DMA path is
    # ramping and every early byte is expensive).
    dmas["w3t0"] = nc.sync.dma_start(
        out=wpk_a[:, 0, 0:1], in_=wpk_r[:, 0, 0:1]
    ).then_inc(pre_sem, 16)
    dmas["x300"] = nc.scalar.dma_start(
        out=apk_a[:, 0, 0, 1], in_=apk_r[:, 0, 0, 1]
    ).then_inc(pre_sem, 16)
    nc.tensor.wait_ge(pre_sem, 32)

    def dep(inst, *keys):
        for k in keys:
            add_dep_helper(inst.ins, dmas[k].ins, sync=True, reason=f"dma {k}")
        return inst

    # Operand views.
    def w3_ap(t, o):
        c = (o % 2) * 256
        return wpk_a[:, t, 0, o // 2, c : c + 256].bitcast(BF)  # [128, 128]

    def w8_ap(t, o):
        return wpk_a[:, t, 1, :, o * 128 : (o + 1) * 128]  # [128, 2, 128]

    def b8_ap(t, n):
        return apk_a[:, t, n, 0]  # [128, 2, 512] fp8 pair

    def x3_ap(t, n):
        return apk_a[:, t, n, 1].bitcast(BF)  # [128, 2, 256] = 512 bf16

    bias_f32 = wb_a[:, 0:16].bitcast(F32)  # [128, 4]

    with tile.TileContext(nc) as tc:
        with (
            tc.tile_pool(name="out", bufs=1) as opool,
            tc.tile_pool(name="psum", bufs=1, space="PSUM") as pspool,
        ):
            y_sb = opool.tile([128, PT, BS], BF)

            # Input DMAs, first thing in the context. Per-queue FIFO data
            # order == emission order (pinned with order-only chains);
            # pieces sized/ordered against the bandwidth ramp and each
            # piece's first-consumer deadline.
            sq = lambda inst: chain("syncq", inst)
            aq = lambda inst: chain("scalq", inst)
            dmas["w8t0"] = sq(
                nc.sync.dma_start(out=wpk_a[:, 0, 1:2], in_=wpk_r[:, 0, 1:2])
            )
            dmas["x301"] = sq(
                nc.sync.dma_start(out=apk_a[:, 0, 1, 1], in_=apk_r[:, 0, 1, 1])
            )
            dmas["b801"] = sq(
                nc.sync.dma_start(out=apk_a[:, 0, 1, 0], in_=apk_r[:, 0, 1, 0])
            )
            dmas["w1"] = sq(nc.sync.dma_start(out=wpk_a[:, 1:2], in_=wpk_r[:, 1:2]))
            dmas["a1"] = sq(nc.sync.dma_start(out=apk_a[:, 1:2], in_=apk_r[:, 1:2]))
            dmas["b800"] = aq(
                nc.scalar.dma_start(out=apk_a[:, 0, 0, 0], in_=apk_r[:, 0, 0, 0])
            )
            dmas["a2"] = aq(nc.scalar.dma_start(out=apk_a[:, 2:3], in_=apk_r[:, 2:3]))
            dmas["w23"] = aq(nc.scalar.dma_start(out=wpk_a[:, 2:PT], in_=wpk_r[:, 2:PT]))
            dmas["a3"] = aq(nc.scalar.dma_start(out=apk_a[:, 3:PT], in_=apk_r[:, 3:PT]))
            dmas["wb"] = aq(nc.scalar.dma_start(out=wb_a, in_=wb_r))

            # 8 PSUM banks: ps[n*4+o] = batch half n, o-tile o, [128, 512] f32
            ps = [
                pspool.tile([128, 512], F32, name=f"ps{i}", tag=f"ps{i}")
                for i in range(2 * PT)
            ]

            def mm_dr(n, o, t, start=False, stop=False):
                # fp8 DoubleRow: contracts the (W1,W2)x(x,x^2) pair (256 deep)
                return pe(
                    nc.tensor.matmul(
                        ps[n * PT + o],
                        w8_ap(t, o),
                        b8_ap(t, n),
                        start=start,
                        stop=stop,
                        perf_mode=mybir.MatmulPerfMode.DoubleRow,
                    )
                )

            def mm_p3(n, o, t, start=False, stop=False):
                return pe(
                    nc.tensor.matmul(
                        ps[n * PT + o],
                        w3_ap(t, o),
                        x3_ap(t, n),
                        start=start,
                        stop=stop,
                    )
                )

            def copy_out(n, o, cs, bank_cs):
                # PSUM -> SBUF: descale (1/S) then add bias (f32 -> bf16)
                return dve(
                    nc.vector.tensor_scalar(
                        y_sb[:, o, cs],
                        ps[n * PT + o][:, bank_cs],
                        1.0 / SCALE,
                        bias_f32[:, o : o + 1],
                        op0=mybir.AluOpType.mult,
                        op1=mybir.AluOpType.add,
                    )
                )

            h0, h1 = slice(0, 512), slice(512, 1024)
            full = slice(0, 512)

            # k-tile 0: ordered to DMA arrival (w-t0 + acts-t0h0 land first,
            # then acts-t0h1).
            mm_p3(0, 0, 0, start=True)
            for o in range(1, PT):
                mm_p3(0, o, 0, start=True)
            dep(mm_dr(0, 0, 0), "w8t0", "b800")
            for o in range(1, PT):
                mm_dr(0, o, 0)
            dep(mm_p3(1, 0, 0, start=True), "x301")
            for o in range(1, PT):
                mm_p3(1, o, 0, start=True)
            dep(mm_dr(1, 0, 0), "b801")
            for o in range(1, PT):
                mm_dr(1, o, 0)
            # k-tile 1: t-major, both halves; paired (n0,n1) shares weights.
            dep(mm_p3(0, 0, 1), "w1", "a1")
            mm_p3(1, 0, 1)
            for o in range(1, PT):
                for n in (0, 1):
                    mm_p3(n, o, 1)
            for o in range(PT):
                for n in (0, 1):
                    mm_dr(n, o, 1)
            # k-tiles 2/3: o-major; each o-group's banks finish and drain
            # under the next group's matmuls.
            for o in range(PT):
                first = mm_dr(0, o, 2)
                if o == 0:
                    dep(first, "w23", "a2")
                mm_dr(1, o, 2)
                d3 = mm_dr(0, o, 3)
                if o == 0:
                    dep(d3, "a3")
                mm_dr(1, o, 3)
                if o < 3:
                    for t in (2, 3):
                        for n in (0, 1):
                            mm_p3(n, o, t, stop=(t == 3))
                    dep(copy_out(0, o, h0, full), *( ["wb"] if o == 0 else [] ))
                    copy_out(1, o, h1, full)
                    queue, qc = [(nc.sync, sq), (nc.scalar, aq)][o % 2]
                    qc(queue.dma_start(out=yt_r[:, o, :], in_=y_sb[:, o, :]))
                else:
                    mm_p3(0, 3, 2)
                    mm_p3(1, 3, 2)
                    # h0 finishes one matmul early: drain + ship it while
                    # the final (h1) matmul runs.
                    mm_p3(0, 3, 3, stop=True)
                    copy_out(0, 3, h0, full)
                    sq(nc.sync.dma_start(out=yt_r[:, 3, h0], in_=y_sb[:, 3, h0]))
                    mm_p3(1, 3, 3, stop=True)
                    copy_out(1, 3, h1, full)
                    aq(nc.scalar.dma_start(out=yt_r[:, 3, h1], in_=y_sb[:, 3, h1]))

    # Post-pass: walrus codegen admits only one sync-wait per instruction
    # encoding here; split any multi-wait instruction into a chain of
    # single-wait drains ahead of it on the same engine queue.
    for bb in nc.m.functions[0].blocks:
        insts = list(bb.instructions)
        out, split = [], 0
        for ins in insts:
            si = ins.sync_info
            waits = list(si.on_wait) if si and si.on_wait else []
            if len(waits) > 1:
                for wx in waits[:-1]:
                    nd = mybir.InstDrain(
                        name=f"drain_split_{split}", engine=ins.engine
                    )
                    split += 1
                    nd.sync_info = mybir.SyncInfo(on_wait=[wx], on_update=[])
                    out.append(nd)
                si.on_wait = [waits[-1]]
            out.append(ins)
        if split:
            bb.set_instructions_from_list(out) if hasattr(
                bb, "set_instructions_from_list"
            ) else setattr(bb, "instructions", out)
    return nc


last_results = None  # BassKernelResults of the most recent run (for test harness)


def kernel(x, edge_w, edge_b, comb_w):
    global _nc, last_results
    if _nc is None:
        _nc = _build()

    bf16 = ml_dtypes.bfloat16
    f8 = ml_dtypes.float8_e4m3
    w_eff = (edge_w * comb_w[:, :, None]).astype(np.float32)  # [I, O, 3]

    # weight pack: [128, t, {0: W3 bytes, 1: (W1,W2) fp8 pair}, 2, 512]
    wpk = np.zeros((128, PT, 2, 2, 512), dtype=f8)
    wpk_u8 = wpk.view(np.uint8)
    for t in range(PT):
        rows = slice(t * 128, (t + 1) * 128)
        w3b = (SCALE * w_eff[rows, :, 2]).astype(bf16)  # [128, 512]
        wpk_u8[:, t, 0] = w3b.view(np.uint8).reshape(128, 2, 512)
        for p in range(2):
            wpk[:, t, 1, p, :] = (SCALE * w_eff[rows, :, p]).astype(f8)
    # bias [128, 4] f32 bit-packed into [128, 16] bytes
    bias = np.sum(comb_w * edge_b, axis=0, dtype=np.float64).astype(np.float32)
    wb_pm = np.zeros((128, 16), dtype=f8)
    wb_pm.view(np.uint8)[:, :] = np.ascontiguousarray(
        bias.reshape(PT, 128).T
    ).view(np.uint8)
    wpk = np.ascontiguousarray(wpk)

    in_maps = []
    for c in range(NCORES):
        xs = np.asarray(x[c * BS : (c + 1) * BS], dtype=np.float32)  # [BS, I]
        # partition-major: [128, 4, 2, 512], [q, t, h, b] = x^T[t*128+q, h*512+b]
        xt = np.ascontiguousarray(xs.T).reshape(PT, 128, BS).transpose(1, 0, 2)
        xh = xt.reshape(128, PT, 2, 512)
        apk = np.zeros((128, PT, 2, 2, 2, 512), dtype=f8)
        apk[:, :, :, 0, 0] = xh.astype(f8)
        apk[:, :, :, 0, 1] = (xh * xh).astype(f8)
        x3b = (xh * xh * xh).astype(bf16)  # [128, PT, 2, 512]
        apk.view(np.uint8)[:, :, :, 1] = x3b.view(np.uint8).reshape(
            128, PT, 2, 2, 512
        )
        in_maps.append(
            {"apk": np.ascontiguousarray(apk), "wpk": wpk, "wb": wb_pm}
        )

    res = bass_utils.run_bass_kernel_spmd(_nc, in_maps, list(range(NCORES)))
    last_results = res
    outs = []
    for c in range(NCORES):
        yt = np.asarray(res.results[c]["yt"])  # [128, 4, 1024] bf16
        outs.append(yt.transpose(1, 0, 2).reshape(O, BS).T.astype(np.float32))
    return np.concatenate(outs, axis=0)


# revision 23
# speedup vs baseline: 1.1336x; 1.1336x over previous
"""KAN layer kernel for Trainium2, data-parallel over 8 NeuronCores.

Math: out[b,o] = sum_i comb_w[i,o] * (w1*x + w2*x^2 + w3*x^3 + edge_b)[b,i,o]
    = x @ W1 + x^2 @ W2 + x^3 @ W3 + bias
  where Wp[i,o] = edge_w[i,o,p] * comb_w[i,o],  bias[o] = sum_i comb_w[i,o]*edge_b[i,o].

Sharding: batch 8-way (1024 rows/core), weights replicated.

Precision: hybrid. x/x^2 terms via fp8-e4m3 DoubleRow (a (W1,W2)x(x,x^2)
pair contracts 256 deep per instruction); the error-dominant x^3 term in
bf16; weights pre-scaled by S=256 (host), descaled in the output copy;
PSUM accumulates fp32. Measured max rel err ~1.13e-2 (gate 2e-2).

All operand prep happens on the HOST (numpy): the fp8 (x,x^2) pair planes
and bf16 x^3 plane are packed, per k-tile and batch-half, into one byte
tensor (apk); W3 (bf16) and the (W1,W2) fp8 pairs pack into another (wpk).
On-device the matmuls read them through bitcast views, so nothing sits
between "bytes land" and "matmul issues".

Schedule (per core):
- Input DMAs are issued BEFORE the TileContext (the context prologue
  otherwise delays the first issue by ~1.3us) and split across both HWDGE
  queues, ordered so each piece lands just before its first consumer:
  sync: w(t0), w(t1), acts(t1), w(t2-3); scalar: bias, acts(t0,h0),
  acts(t0,h1), acts(t2), acts(t3). Early pieces are small (the DMA path
  ramps from ~150 GB/s to ~390 GB/s over the first ~4us). Consumers hang
  off manual sync deps (the tensors are raw SBUF, not pool tiles, so
  cross-queue pieces never serialize on tile bookkeeping).
- ~2us of dummy matmuls bridge the PE from the preamble to the first
  operands so the HAM clock gate stays at 8/8 (2.4 GHz); an idle gap
  >~3us here re-throttles the PE to half clock for several us (measured).
- 64 matmuls, one dense stream, 8 PSUM banks = (batch half, o-tile).
  k-tiles 0/1 t-major, ordered to match DMA arrival (p3[h0], dr[h0],
  p3[h1], dr[h1] for t0); k-tiles 2/3 o-major so each bank finishes early
  and drains (DVE descale+bias -> bf16, then y DMA) under the next
  o-group's matmuls. The last o-tile drains h0 while the final matmul
  (h1) runs, h1 immediately after, on separate queues.
"""

import sys

import numpy as np
import ml_dtypes

sys.path.insert(0, "/opt/trn_rl_repo")

import concourse.bass as bass
import concourse.tile as tile
from concourse import bass_utils, mybir
from concourse.tile_rust import add_dep_helper

B, I, O = 8192, 512, 512
NCORES = 8
BS = B // NCORES  # 1024 rows per core
PT = 4  # 128-row tiles in I (k-tiles) and O (o-tiles)
SCALE = 256.0  # fp8/bf16 weight pre-scale (host); descaled in the output copy

BF = mybir.dt.bfloat16
F8 = mybir.dt.float8e4
F32 = mybir.dt.float32

_nc = None


def _build():
    # All HBM tensors are partition-major (leading dim 128 = SBUF partition).
    # wpk[q, t, 0] = W3 row bytes (512 bf16 as [2,512] u8); wpk[q, t, 1, p] =
    # fp8 Wp pair plane. apk[q, t, h, 0, p] = fp8 (x, x^2) pair planes;
    # apk[q, t, h, 1] = x^3 bytes (512 bf16 as [2,512] u8).
    nc = bass.Bass("TRN2", target_bir_lowering=False, debug=False)
    wpkt = nc.dram_tensor("wpk", [128, PT, 2, 2, 512], F8, kind="ExternalInput")
    apkt = nc.dram_tensor("apk", [128, PT, 2, 2, 2, 512], F8, kind="ExternalInput")
    wbt = nc.dram_tensor("wb", [128, 16], F8, kind="ExternalInput")
    yt = nc.dram_tensor("yt", [128, PT, BS], BF, kind="ExternalOutput")
    wpk_r, apk_r, wb_r, yt_r = wpkt.ap(), apkt.ap(), wbt.ap(), yt.ap()

    chains = {}

    def chain(key, inst):
        prev = chains.get(key)
        if prev is not None:
            add_dep_helper(inst.ins, prev.ins, sync=False, reason=f"{key} order")
        chains[key] = inst
        return inst

    pe = lambda inst: chain("pe", inst)
    dve = lambda inst: chain("dve", inst)

    # Raw SBUF operand buffers (not pool tiles): consumers are wired up with
    # explicit deps below, so DMA pieces never serialize on tile bookkeeping.
    wpk_sb = nc.alloc_sbuf_tensor("wpk_sb", [128, PT, 2, 2, 512], F8)
    apk_sb = nc.alloc_sbuf_tensor("apk_sb", [128, PT, 2, 2, 2, 512], F8)
    wb_sb = nc.alloc_sbuf_tensor("wb_sb", [128, 16], F8)
    wpk_a, apk_a, wb_a = wpk_sb.ap(), apk_sb.ap(), wb_sb.ap()

    # HAM warm-up: dummy matmuls on garbage SBUF bridge the PE from the
    # engine preamble (~6.9us) to the first operands (~8.5us) so the clock
    # gate is 8/8 when real matmuls start. Scratch PSUM is freed before the
    # pools allocate; real banks are zeroed by their start=True matmuls.
    warm_w = nc.alloc_sbuf_tensor("warm_w", [128, 2], BF)
    warm_x = nc.alloc_sbuf_tensor("warm_x", [128, 512], BF)
    with nc.psum_tensor("warm_ps", [128, 512], F32) as wps:
        for i in range(9):
            nc.tensor.matmul(
                wps.ap()[0:2, :], warm_w.ap(), warm_x.ap(), start=True, stop=True
            )
        for i in range(6):
            nc.tensor.matmul(
                wps.ap()[0:2, 0:128],
                warm_w.ap(),
                warm_x.ap()[:, 0:128],
                start=True,
                stop=True,
            )

    # The two pieces gating the first matmuls go out BEFORE the tile
    # context (the context prologue delays the first in-context issue by
    # ~1.4us, and the DMA path ramps slowly — start it early). They carry
    # an explicit completion semaphore; a PE-queue wait after the warmup
    # matmuls gates every real matmul behind both pieces (PE executes its
    # queue in order).
    dmas = {}
    pre_sem = nc.alloc_semaphore("pre_dma")
    nc.sync.sem_clear(pre_sem)
    # First pieces: all of (t0 weights) and (t0,h0) activations — exactly
    # what the first eight matmuls read. Kept at 256 KB: finer slicing
    # exposes more completion-sem boundaries to DMA-engine straggler skew
    # (a lagging engine can hold a piece's sem for ~1-3us).
    dmas["w0"] = nc.sync.dma_start(out=wpk_a[:, 0:1], in_=wpk_r[:, 0:1]).then_inc(
        pre_sem, 16
    )
    dmas["a00"] = nc.scalar.dma_start(
        out=apk_a[:, 0, 0], in_=apk_r[:, 0, 0]
    ).then_inc(pre_sem, 16)
    nc.tensor.wait_ge(pre_sem, 32)

    def dep(inst, *keys):
        for k in keys:
            add_dep_helper(inst.ins, dmas[k].ins, sync=True, reason=f"dma {k}")
        return inst

    # Operand views.
    def w3_ap(t, o):
        c = (o % 2) * 256
        return wpk_a[:, t, 0, o // 2, c : c + 256].bitcast(BF)  # [128, 128]

    def w8_ap(t, o):
        return wpk_a[:, t, 1, :, o * 128 : (o + 1) * 128]  # [128, 2, 128]

    def b8_ap(t, n):
        return apk_a[:, t, n, 0]  # [128, 2, 512] fp8 pair

    def x3_ap(t, n):
        return apk_a[:, t, n, 1].bitcast(BF)  # [128, 2, 256] = 512 bf16

    bias_f32 = wb_a[:, 0:16].bitcast(F32)  # [128, 4]

    with tile.TileContext(nc) as tc:
        with (
            tc.tile_pool(name="out", bufs=1) as opool,
            tc.tile_pool(name="psum", bufs=1, space="PSUM") as pspool,
        ):
            y_sb = opool.tile([128, PT, BS], BF)

            # Input DMAs, first thing in the context. Per-queue FIFO data
            # order == emission order (pinned with order-only chains);
            # pieces sized/ordered against the bandwidth ramp and each
            # piece's first-consumer deadline.
            sq = lambda inst: chain("syncq", inst)
            aq = lambda inst: chain("scalq", inst)
            dmas["w123"] = sq(
                nc.sync.dma_start(out=wpk_a[:, 1:PT], in_=wpk_r[:, 1:PT])
            )
            dmas["a2"] = sq(nc.sync.dma_start(out=apk_a[:, 2:3], in_=apk_r[:, 2:3]))
            dmas["a01"] = aq(nc.scalar.dma_start(out=apk_a[:, 0, 1], in_=apk_r[:, 0, 1]))
            dmas["a1"] = aq(nc.scalar.dma_start(out=apk_a[:, 1:2], in_=apk_r[:, 1:2]))
            dmas["a3"] = aq(nc.scalar.dma_start(out=apk_a[:, 3:PT], in_=apk_r[:, 3:PT]))
            dmas["wb"] = aq(nc.scalar.dma_start(out=wb_a, in_=wb_r))

            # 8 PSUM banks: ps[n*4+o] = batch half n, o-tile o, [128, 512] f32
            ps = [
                pspool.tile([128, 512], F32, name=f"ps{i}", tag=f"ps{i}")
                for i in range(2 * PT)
            ]

            def mm_dr(n, o, t, start=False, stop=False):
                # fp8 DoubleRow: contracts the (W1,W2)x(x,x^2) pair (256 deep)
                return pe(
                    nc.tensor.matmul(
                        ps[n * PT + o],
                        w8_ap(t, o),
                        b8_ap(t, n),
                        start=start,
                        stop=stop,
                        perf_mode=mybir.MatmulPerfMode.DoubleRow,
                    )
                )

            def mm_p3(n, o, t, start=False, stop=False):
                return pe(
                    nc.tensor.matmul(
                        ps[n * PT + o],
                        w3_ap(t, o),
                        x3_ap(t, n),
                        start=start,
                        stop=stop,
                    )
                )

            def copy_out(n, o, cs, bank_cs):
                # PSUM -> SBUF: descale (1/S) then add bias (f32 -> bf16)
                return dve(
                    nc.vector.tensor_scalar(
                        y_sb[:, o, cs],
                        ps[n * PT + o][:, bank_cs],
                        1.0 / SCALE,
                        bias_f32[:, o : o + 1],
                        op0=mybir.AluOpType.mult,
                        op1=mybir.AluOpType.add,
                    )
                )

            h0, h1 = slice(0, 512), slice(512, 1024)
            full = slice(0, 512)

            # k-tile 0: ordered to DMA arrival (w-t0 + acts-t0h0 land first,
            # then acts-t0h1).
            mm_p3(0, 0, 0, start=True)
            for o in range(1, PT):
                mm_p3(0, o, 0, start=True)
            for o in range(PT):
                mm_dr(0, o, 0)
            dep(mm_p3(1, 0, 0, start=True), "a01")
            for o in range(1, PT):
                mm_p3(1, o, 0, start=True)
            for o in range(PT):
                mm_dr(1, o, 0)
            # k-tile 1: t-major, both halves; paired (n0,n1) shares weights.
            dep(mm_p3(0, 0, 1), "w123", "a1")
            mm_p3(1, 0, 1)
            for o in range(1, PT):
                for n in (0, 1):
                    mm_p3(n, o, 1)
            for o in range(PT):
                for n in (0, 1):
                    mm_dr(n, o, 1)
            # k-tiles 2/3: o-major; each o-group's banks finish and drain
            # under the next group's matmuls.
            for o in range(PT):
                first = mm_dr(0, o, 2)
                if o == 0:
                    dep(first, "a2")
                mm_dr(1, o, 2)
                d3 = mm_dr(0, o, 3)
                if o == 0:
                    dep(d3, "a3")
                mm_dr(1, o, 3)
                if o < 3:
                    for t in (2, 3):
                        for n in (0, 1):
                            mm_p3(n, o, t, stop=(t == 3))
                    dep(copy_out(0, o, h0, full), *( ["wb"] if o == 0 else [] ))
                    copy_out(1, o, h1, full)
                    queue, qc = [(nc.sync, sq), (nc.scalar, aq)][o % 2]
                    qc(queue.dma_start(out=yt_r[:, o, :], in_=y_sb[:, o, :]))
                else:
                    mm_p3(0, 3, 2)
                    mm_p3(1, 3, 2)
                    # h0 finishes one matmul early: drain + ship it while
                    # the final (h1) matmul runs.
                    mm_p3(0, 3, 3, stop=True)
                    copy_out(0, 3, h0, full)
                    sq(nc.sync.dma_start(out=yt_r[:, 3, h0], in_=y_sb[:, 3, h0]))
                    mm_p3(1, 3, 3, stop=True)
                    copy_out(1, 3, h1, full)
                    aq(nc.scalar.dma_start(out=yt_r[:, 3, h1], in_=y_sb[:, 3, h1]))

    # Post-pass: walrus codegen admits only one sync-wait per instruction
    # encoding here; split any multi-wait instruction into a chain of
    # single-wait drains ahead of it on the same engine queue.
    for bb in nc.m.functions[0].blocks:
        insts = list(bb.instructions)
        out, split = [], 0
        for ins in insts:
            si = ins.sync_info
            waits = list(si.on_wait) if si and si.on_wait else []
            if len(waits) > 1:
                for wx in waits[:-1]:
                    nd = mybir.InstDrain(
                        name=f"drain_split_{split}", engine=ins.engine
                    )
                    split += 1
                    nd.sync_info = mybir.SyncInfo(on_wait=[wx], on_update=[])
                    out.append(nd)
                si.on_wait = [waits[-1]]
            out.append(ins)
        if split:
            bb.set_instructions_from_list(out) if hasattr(
                bb, "set_instructions_from_list"
            ) else setattr(bb, "instructions", out)
    return nc


last_results = None  # BassKernelResults of the most recent run (for test harness)


def kernel(x, edge_w, edge_b, comb_w):
    global _nc, last_results
    if _nc is None:
        _nc = _build()

    bf16 = ml_dtypes.bfloat16
    f8 = ml_dtypes.float8_e4m3
    w_eff = (edge_w * comb_w[:, :, None]).astype(np.float32)  # [I, O, 3]

    # weight pack: [128, t, {0: W3 bytes, 1: (W1,W2) fp8 pair}, 2, 512]
    wpk = np.zeros((128, PT, 2, 2, 512), dtype=f8)
    wpk_u8 = wpk.view(np.uint8)
    for t in range(PT):
        rows = slice(t * 128, (t + 1) * 128)
        w3b = (SCALE * w_eff[rows, :, 2]).astype(bf16)  # [128, 512]
        wpk_u8[:, t, 0] = w3b.view(np.uint8).reshape(128, 2, 512)
        for p in range(2):
            wpk[:, t, 1, p, :] = (SCALE * w_eff[rows, :, p]).astype(f8)
    # bias [128, 4] f32 bit-packed into [128, 16] bytes
    bias = np.sum(comb_w * edge_b, axis=0, dtype=np.float64).astype(np.float32)
    wb_pm = np.zeros((128, 16), dtype=f8)
    wb_pm.view(np.uint8)[:, :] = np.ascontiguousarray(
        bias.reshape(PT, 128).T
    ).view(np.uint8)
    wpk = np.ascontiguousarray(wpk)

    in_maps = []
    for c in range(NCORES):
        xs = np.asarray(x[c * BS : (c + 1) * BS], dtype=np.float32)  # [BS, I]
        # partition-major: [128, 4, 2, 512], [q, t, h, b] = x^T[t*128+q, h*512+b]
        xt = np.ascontiguousarray(xs.T).reshape(PT, 128, BS).transpose(1, 0, 2)
        xh = xt.reshape(128, PT, 2, 512)
        apk = np.zeros((128, PT, 2, 2, 2, 512), dtype=f8)
        apk[:, :, :, 0, 0] = xh.astype(f8)
        apk[:, :, :, 0, 1] = (xh * xh).astype(f8)
        x3b = (xh * xh * xh).astype(bf16)  # [128, PT, 2, 512]
        apk.view(np.uint8)[:, :, :, 1] = x3b.view(np.uint8).reshape(
            128, PT, 2, 2, 512
        )
        in_maps.append(
            {"apk": np.ascontiguousarray(apk), "wpk": wpk, "wb": wb_pm}
        )

    res = bass_utils.run_bass_kernel_spmd(_nc, in_maps, list(range(NCORES)))
    last_results = res
    outs = []
    for c in range(NCORES):
        yt = np.asarray(res.results[c]["yt"])  # [128, 4, 1024] bf16
        outs.append(yt.transpose(1, 0, 2).reshape(O, BS).T.astype(np.float32))
    return np.concatenate(outs, axis=0)
